# revision 20
# baseline (speedup 1.0000x reference)
"""Trainium2 Bass kernel for nn_Decoder_13331578487330.

Model (reference semantics, all per batch-block):
  blocks[bs,na,na] -> x = blocks @ W_sp.T + b_sp        [bs,na,64]
  30x: 2-layer LSTM single step with zero initial state (so the f-gate
       contributes nothing: c = sig(i)*tanh(g), h = sig(o)*tanh(c)),
       emb = h @ W_h2p.T + b_h2p (the scan carry),
       rel = relu(emb @ W_m1.T + b_m1) @ W_m2.T + b_m2  -> traj[s]
  h_final is always zeros.

Sharding: data-parallel over the 128 diagonal blocks; 16 blocks (1024
rows) per NeuronCore, weights replicated. No cross-core communication.

On-chip layout: activations stored transposed ([channels, rows]) so
weights are the stationary matmul operand and per-channel biases land on
partitions (fused into ScalarE activation ops). Compute dtype bf16 with
fp32 PSUM accumulation (measured l2 rel err ~1e-3 vs fp32 reference).
"""

import os
import sys

import numpy as np

for _p in ("/opt/trn_rl_repo",):
    if _p not in sys.path and os.path.isdir(_p):
        sys.path.insert(0, _p)

import ml_dtypes

B, NA, S, H, EMB, MLP_H, NL = 128, 64, 30, 512, 64, 64, 2
N_CORES = 8
BS_LOCAL = B // N_CORES          # 16 diagonal blocks per core
ROWS = BS_LOCAL * NA             # 1024 rows per core
G3 = 3 * H                       # i,g,o gates only (f is dead)
QT = G3 // 128                   # 12 gate channel tiles of 128
KT = H // 128                    # 4 contraction tiles for H=512
NCH = ROWS // 512                # 2 row chunks of 512 (fp32 psum bank)

BF16 = ml_dtypes.bfloat16

# deg-5 odd minimax-ish fit of tanh on [-0.9, 0.9] (|c| <= ~0.54 in practice;
# max approx err 4.8e-4, below the bf16 noise floor)
TANH_A0, TANH_A1, TANH_A2 = 0.99861711, -0.31700099, 0.08317692

_cache = {}


def _bf(a):
    return np.ascontiguousarray(np.asarray(a, np.float32).astype(BF16))


def _build_program():
    import concourse.bass as bass
    import concourse.tile as tile
    from concourse import mybir

    f32 = mybir.dt.float32
    bf16 = mybir.dt.bfloat16
    AF = mybir.ActivationFunctionType

    nc = bass.Bass()

    blk_d = nc.declare_dram_parameter("blk", [NA, ROWS], bf16, isOutput=False)
    w0_d = nc.declare_dram_parameter("w0", [EMB, G3], bf16, isOutput=False)
    w1_d = nc.declare_dram_parameter("w1", [KT, 128, G3], bf16, isOutput=False)
    wh2p_d = nc.declare_dram_parameter("wh2p", [KT, 128, EMB], bf16, isOutput=False)
    wm1_d = nc.declare_dram_parameter("wm1", [EMB, MLP_H], bf16, isOutput=False)
    wm2_d = nc.declare_dram_parameter("wm2", [MLP_H + 2, 2], bf16, isOutput=False)
    wsp_d = nc.declare_dram_parameter("wsp", [NA, EMB], bf16, isOutput=False)
    bias_d = nc.declare_dram_parameter("bias", [128, 32], f32, isOutput=False)
    traj_d = nc.declare_dram_parameter("traj", [S, ROWS, 2], f32, isOutput=True)

    from contextlib import ExitStack

    with tile.TileContext(nc) as tc, ExitStack() as ctx:
        wp = ctx.enter_context(tc.tile_pool(name="wp", bufs=1))
        st = ctx.enter_context(tc.tile_pool(name="st", bufs=1))
        tmp = ctx.enter_context(tc.tile_pool(name="tmp", bufs=4))
        relp = ctx.enter_context(tc.tile_pool(name="relp", bufs=4))
        gp = ctx.enter_context(tc.tile_pool(name="gp", bufs=3, space="PSUM"))
        mp = ctx.enter_context(tc.tile_pool(name="mp", bufs=1, space="PSUM"))
        scr = ctx.enter_context(tc.tile_pool(name="scr", bufs=1, space="PSUM"))

        # --- resident weights/biases ---
        w0s = wp.tile([EMB, G3], bf16, tag="w0")
        nc.sync.dma_start(w0s[:], w0_d[:])
        w1s = []
        for k in range(KT):
            t = wp.tile([128, G3], bf16, tag=f"w1_{k}")
            nc.sync.dma_start(t[:], w1_d[k])
            w1s.append(t)
        wh2ps = []
        for k in range(KT):
            t = wp.tile([128, EMB], bf16, tag=f"wh2p_{k}")
            nc.sync.dma_start(t[:], wh2p_d[k])
            wh2ps.append(t)
        wm1s = wp.tile([EMB, MLP_H], bf16, tag="wm1")
        nc.sync.dma_start(wm1s[:], wm1_d[:])
        wm2s = wp.tile([MLP_H + 2, 2], bf16, tag="wm2")
        nc.sync.dma_start(wm2s[:], wm2_d[:])
        wsps = wp.tile([NA, EMB], bf16, tag="wsp")
        nc.sync.dma_start(wsps[:], wsp_d[:])
        bs_ = wp.tile([128, 32], f32, tag="bias")
        nc.sync.dma_start(bs_[:], bias_d[:])
        blks = st.tile([NA, ROWS], bf16, tag="blk")
        nc.sync.dma_start(blks[:], blk_d[:])

        # --- persistent state ---
        embT = st.tile([EMB, ROWS], bf16, tag="embT")
        h0T = [st.tile([128, ROWS], bf16, tag=f"h0T_{k}", name=f"h0T_{k}")
               for k in range(KT)]
        h1T = [st.tile([128, ROWS], bf16, tag=f"h1T_{k}", name=f"h1T_{k}")
               for k in range(KT)]
        tT = st.tile([MLP_H + 2, ROWS], bf16, tag="tT")
        nc.vector.memset(tT[MLP_H:MLP_H + 2, :], 1.0)  # ones rows for m2 bias

        # scratch psum for keep-warm matmuls: the HAM clock gate halves the
        # PE clock after ~3.4us of idle; the ACT-bound L0 phase idles PE
        # long enough to re-throttle every step (25% of matmuls measured at
        # 1.2 GHz). Dummy matmuls spaced through that phase keep it at 2.4.
        ps_warm = scr.tile([EMB, 512], f32, tag="warm")

        def warm():
            nc.tensor.matmul(ps_warm[:], wsps[:], blks[:, 0:512],
                             start=True, stop=True)

        # --- initial spatial embedding: embT = (blocks @ W_sp.T + b_sp).T ---
        for n in range(NCH):
            sl = slice(n * 512, (n + 1) * 512)
            ps0 = mp.tile([EMB, 512], f32, tag="mp", name=f"ps0_{n}")
            nc.tensor.matmul(ps0[:], wsps[:], blks[:, sl], start=True, stop=True)
            nc.vector.tensor_scalar_add(embT[:, sl], ps0[:], bs_[0:EMB, 24:25])

        def emit_tail(s):
            # --- t = relu(emb @ W_m1.T + b_m1); bias+relu fused on DVE ---
            for n in range(NCH):
                sl = slice(n * 512, (n + 1) * 512)
                ps_m = mp.tile([MLP_H, 512], f32, tag="mp", name=f"ps_m_{s}_{n}")
                nc.tensor.matmul(ps_m[:], wm1s[:], embT[:, sl],
                                 start=True, stop=True)
                nc.vector.tensor_scalar(tT[0:MLP_H, sl], ps_m[:],
                                        bs_[0:MLP_H, 26:27], 0.0,
                                        mybir.AluOpType.add, mybir.AluOpType.max)

            # --- rel = t @ W_m2.T + b_m2 (bias via ones rows), rows on psum.
            # All 8 row-tiles land in one psum tile, one copy, one DMA. ---
            m2big = gp.tile([128, 16], f32, tag="gps", name=f"m2big_{s}")
            for r in range(ROWS // 128):
                nc.tensor.matmul(m2big[:, r * 2:(r + 1) * 2],
                                 tT[:, r * 128:(r + 1) * 128], wm2s[:],
                                 start=True, stop=True)
            warm()
            rstage = relp.tile([128, 16], f32, tag="rel")
            nc.vector.tensor_copy(rstage[:], m2big[:])
            out_ap = traj_d[s].rearrange("(r p) k -> p r k", p=128)
            nc.sync.dma_start(out_ap, rstage[:].rearrange("p (r k) -> p r k", k=2))

        for s in range(S):
            # --- 2-layer LSTM cell, zero state ---
            for layer in range(2):
                boff = 12 * layer
                hdst = h0T if layer == 0 else h1T

                def gate_mms(ps, gi, q):
                    # k outermost so one weight load covers both row chunks
                    col = (gi * 4 + q) * 128
                    if layer == 0:
                        for n in range(NCH):
                            sl = slice(n * 512, (n + 1) * 512)
                            nc.tensor.matmul(
                                ps[:, sl], w0s[:, col:col + 128], embT[:, sl],
                                start=True, stop=True)
                    else:
                        for k in range(KT):
                            for n in range(NCH):
                                sl = slice(n * 512, (n + 1) * 512)
                                nc.tensor.matmul(
                                    ps[:, sl], w1s[k][:, col:col + 128],
                                    h0T[k][:, sl],
                                    start=(k == 0), stop=(k == KT - 1))

                for q in range(4):
                    ps_i = gp.tile([128, ROWS], f32, tag="gps")
                    gate_mms(ps_i, 0, q)
                    ps_g = gp.tile([128, ROWS], f32, tag="gps")
                    gate_mms(ps_g, 1, q)
                    si = tmp.tile([128, ROWS], bf16, tag="si")
                    nc.scalar.activation(si[:], ps_i[:], AF.Sigmoid,
                                         bias=bs_[:, boff + q:boff + q + 1])
                    tg = tmp.tile([128, ROWS], bf16, tag="tg")
                    nc.scalar.activation(tg[:], ps_g[:], AF.Tanh,
                                         bias=bs_[:, boff + 4 + q:boff + 5 + q])
                    cc = tmp.tile([128, ROWS], bf16, tag="cc")
                    nc.vector.tensor_mul(cc[:], si[:], tg[:])
                    ps_o = gp.tile([128, ROWS], f32, tag="gps")
                    gate_mms(ps_o, 2, q)
                    so = tmp.tile([128, ROWS], bf16, tag="so")
                    nc.scalar.activation(so[:], ps_o[:], AF.Sigmoid,
                                         bias=bs_[:, boff + 8 + q:boff + 9 + q])
                    if layer == 0:
                        # ACT has slack during the L0 phase; keep the short
                        # ACT chain here so h0 lands sooner.
                        tc_ = tmp.tile([128, ROWS], bf16, tag="tc")
                        nc.scalar.activation(tc_[:], cc[:], AF.Tanh)
                        nc.vector.tensor_mul(hdst[q][:], so[:], tc_[:])
                        warm()
                    else:
                        # h = tanh(c)*sig(o) via deg-5 odd poly on DVE:
                        # h = (A0 + A1 c^2 + A2 c^4) * c * sig(o)
                        c2 = tmp.tile([128, ROWS], bf16, tag="c2")
                        nc.vector.tensor_mul(c2[:], cc[:], cc[:])
                        pp = tmp.tile([128, ROWS], bf16, tag="pp")
                        nc.vector.tensor_scalar(pp[:], c2[:], TANH_A2, TANH_A1,
                                                mybir.AluOpType.mult,
                                                mybir.AluOpType.add)
                        vv = tmp.tile([128, ROWS], bf16, tag="vv")
                        nc.vector.tensor_mul(vv[:], pp[:], c2[:])
                        ww = tmp.tile([128, ROWS], bf16, tag="ww")
                        nc.vector.tensor_mul(ww[:], cc[:], so[:])
                        nc.vector.scalar_tensor_tensor(
                            hdst[q][:], vv[:], TANH_A0, ww[:],
                            mybir.AluOpType.add, mybir.AluOpType.mult)

                if layer == 0 and s > 0:
                    # step s-1's MLP head: real PE work placed inside the
                    # ACT-bound L0 phase where the PE would otherwise idle
                    emit_tail(s - 1)

            # --- emb = h1 @ W_h2p.T + b_h2p (carry); bias-add on DVE ---
            for n in range(NCH):
                sl = slice(n * 512, (n + 1) * 512)
                ps_e = mp.tile([EMB, 512], f32, tag="mp", name=f"ps_e_{s}_{n}")
                for k in range(KT):
                    nc.tensor.matmul(ps_e[:], wh2ps[k][:], h1T[k][:, sl],
                                     start=(k == 0), stop=(k == KT - 1))
                nc.vector.tensor_scalar_add(embT[:, sl], ps_e[:],
                                            bs_[0:EMB, 25:26])

        emit_tail(S - 1)

    _split_excess_waits(nc)
    return nc


def _split_excess_waits(nc, max_waits=1):
    """walrus rejects instructions carrying more than one semaphore wait
    (seen on the Tile kernel-tail drain). Move excess waits onto preceding
    same-engine NOPs — engines execute in order, so semantics hold."""
    from concourse import mybir

    cnt = 0
    for f in nc.m.functions:
        for blk in f.blocks:
            il = list(blk.instructions)
            changed = False
            new = []
            for ins in il:
                si = getattr(ins, "sync_info", None)
                if si is not None and len(si.on_wait) > max_waits:
                    waits = list(si.on_wait)
                    while len(waits) > max_waits:
                        chunk, waits = waits[:max_waits], waits[max_waits:]
                        nop = mybir.InstNoOp(name=f"wsplit_{cnt}", ins=[], outs=[])
                        cnt += 1
                        nop.engine = ins.engine
                        nop.sync_info = mybir.SyncInfo(on_wait=chunk, on_update=[])
                        new.append(nop)
                    ins.sync_info = mybir.SyncInfo(
                        on_wait=waits, on_update=list(si.on_update))
                    changed = True
                new.append(ins)
            if changed:
                blk.instructions = new
    return cnt


def _install_ntff_hook():
    """The agent image lacks antenv.axon_hooks; synthesize it so
    run_bass_kernel_spmd(trace=True) can capture NTFF profiles."""
    import types

    if "antenv.axon_hooks" in sys.modules:
        return
    try:
        import trn_agent_boot.trn_boot as tb
        hook = tb._ntff_profile_via_ctypes("/opt/axon/libaxon_pjrt.so")
    except Exception:
        hook = None
    m = types.ModuleType("antenv.axon_hooks")
    m.get_axon_ntff_profile_hook = lambda: hook
    sys.modules["antenv.axon_hooks"] = m


def _prep_inputs(inputs):
    """Shard + lay out host-side: per-core input maps."""
    ind = np.asarray(inputs["input_data"], np.float32)
    n = ind.shape[0]
    assert n == B * NA, ind.shape
    idx = np.arange(B)
    blocks = ind.reshape(B, NA, B, NA)[idx, :, idx, :]  # [B, na, na]

    gate_cols = np.r_[0:H, 2 * H:4 * H]  # i, g, o rows of [4H, *] weights

    w0 = _bf(np.asarray(inputs["W_ih0"], np.float32).T[:, gate_cols])  # [64,1536]
    w1 = _bf(np.asarray(inputs["W_ih1"], np.float32).T[:, gate_cols]
              .reshape(KT, 128, G3))
    wh2p = _bf(np.asarray(inputs["W_h2p"], np.float32).T.reshape(KT, 128, EMB))
    wm1 = _bf(np.asarray(inputs["W_m1"], np.float32).T)  # [64,64]
    wsp = _bf(np.asarray(inputs["W_sp"], np.float32).T)  # [na,EMB]

    b_m2 = np.asarray(inputs["b_m2"], np.float32)
    bm2_hi = b_m2.astype(BF16).astype(np.float32)
    bm2_lo = b_m2 - bm2_hi
    wm2 = np.concatenate([
        np.asarray(inputs["W_m2"], np.float32).T,  # [64,2]
        bm2_hi[None, :], bm2_lo[None, :]], axis=0)
    wm2 = _bf(wm2)  # [66,2]

    bias = np.zeros((128, 32), np.float32)
    b0 = (np.asarray(inputs["b_ih0"], np.float32)
          + np.asarray(inputs["b_hh0"], np.float32))[gate_cols]
    b1 = (np.asarray(inputs["b_ih1"], np.float32)
          + np.asarray(inputs["b_hh1"], np.float32))[gate_cols]
    bias[:, 0:12] = b0.reshape(12, 128).T
    bias[:, 12:24] = b1.reshape(12, 128).T
    bias[0:EMB, 24] = np.asarray(inputs["b_sp"], np.float32)
    bias[0:EMB, 25] = np.asarray(inputs["b_h2p"], np.float32)
    bias[0:MLP_H, 26] = np.asarray(inputs["b_m1"], np.float32)

    shared = dict(w0=w0, w1=w1, wh2p=wh2p, wm1=wm1, wm2=wm2, wsp=wsp, bias=bias)
    in_maps = []
    for c in range(N_CORES):
        blk = blocks[c * BS_LOCAL:(c + 1) * BS_LOCAL]          # [16,na,na]
        blkT = _bf(blk.transpose(2, 0, 1).reshape(NA, ROWS))    # [na,1024]
        in_maps.append(dict(shared, blk=blkT))
    return in_maps


last_results = None


def kernel(input_data, num_agents, W_sp, b_sp, W_ih0, b_ih0, b_hh0,
           W_ih1, b_ih1, b_hh1, W_h2p, b_h2p, W_m1, b_m1, W_m2, b_m2):
    global last_results
    from concourse.bass_utils import run_bass_kernel_spmd

    inputs = dict(input_data=input_data, W_sp=W_sp, b_sp=b_sp,
                  W_ih0=W_ih0, b_ih0=b_ih0, b_hh0=b_hh0,
                  W_ih1=W_ih1, b_ih1=b_ih1, b_hh1=b_hh1,
                  W_h2p=W_h2p, b_h2p=b_h2p, W_m1=W_m1, b_m1=b_m1,
                  W_m2=W_m2, b_m2=b_m2)

    if "nc" not in _cache:
        _cache["nc"] = _build_program()
    nc = _cache["nc"]

    in_maps = _prep_inputs(inputs)
    trace = bool(int(os.environ.get("KERNEL_TRACE", "0")))
    if trace:
        _install_ntff_hook()
    res = run_bass_kernel_spmd(nc, in_maps, list(range(N_CORES)), trace=trace)
    last_results = res

    traj = np.empty((S, B * NA, 2), np.float32)
    for c in range(N_CORES):
        traj[:, c * ROWS:(c + 1) * ROWS, :] = res.results[c]["traj"]
    h_final = np.zeros((NL, NA, H), np.float32)
    return traj, h_final


# revision 24
# speedup vs baseline: 1.0603x; 1.0603x over previous
"""Trainium2 Bass kernel for nn_Decoder_13331578487330.

Model (reference semantics, all per batch-block):
  blocks[bs,na,na] -> x = blocks @ W_sp.T + b_sp        [bs,na,64]
  30x: 2-layer LSTM single step with zero initial state (so the f-gate
       contributes nothing: c = sig(i)*tanh(g), h = sig(o)*tanh(c)),
       emb = h @ W_h2p.T + b_h2p (the scan carry),
       rel = relu(emb @ W_m1.T + b_m1) @ W_m2.T + b_m2  -> traj[s]
  h_final is always zeros.

Sharding: data-parallel over the 128 diagonal blocks; 16 blocks (1024
rows) per NeuronCore, weights replicated. No cross-core communication.

On-chip layout: activations stored transposed ([channels, rows]) so
weights are the stationary matmul operand and per-channel biases land on
partitions (fused into ScalarE activation ops). Compute dtype bf16 with
fp32 PSUM accumulation (measured l2 rel err ~1e-3 vs fp32 reference).
"""

import os
import sys

import numpy as np

for _p in ("/opt/trn_rl_repo",):
    if _p not in sys.path and os.path.isdir(_p):
        sys.path.insert(0, _p)

import ml_dtypes

B, NA, S, H, EMB, MLP_H, NL = 128, 64, 30, 512, 64, 64, 2
N_CORES = 8
BS_LOCAL = B // N_CORES          # 16 diagonal blocks per core
ROWS = BS_LOCAL * NA             # 1024 rows per core
G3 = 3 * H                       # i,g,o gates only (f is dead)
QT = G3 // 128                   # 12 gate channel tiles of 128
KT = H // 128                    # 4 contraction tiles for H=512
NCH = ROWS // 512                # 2 row chunks of 512 (fp32 psum bank)

BF16 = ml_dtypes.bfloat16

# deg-5 odd minimax-ish fit of tanh on [-0.9, 0.9] (|c| <= ~0.54 in practice;
# max approx err 4.8e-4, below the bf16 noise floor)
TANH_A0, TANH_A1, TANH_A2 = 0.99861711, -0.31700099, 0.08317692

_cache = {}


def _bf(a):
    return np.ascontiguousarray(np.asarray(a, np.float32).astype(BF16))


def _build_program():
    import concourse.bass as bass
    import concourse.tile as tile
    from concourse import mybir

    f32 = mybir.dt.float32
    bf16 = mybir.dt.bfloat16
    AF = mybir.ActivationFunctionType

    nc = bass.Bass()

    blk_d = nc.declare_dram_parameter("blk", [NA, ROWS], bf16, isOutput=False)
    w0_d = nc.declare_dram_parameter("w0", [EMB, G3], bf16, isOutput=False)
    w1_d = nc.declare_dram_parameter("w1", [KT, 128, G3], bf16, isOutput=False)
    wh2p_d = nc.declare_dram_parameter("wh2p", [KT, 128, EMB], bf16, isOutput=False)
    wm1_d = nc.declare_dram_parameter("wm1", [EMB, MLP_H], bf16, isOutput=False)
    wm2_d = nc.declare_dram_parameter("wm2", [MLP_H + 2, 2], bf16, isOutput=False)
    wsp_d = nc.declare_dram_parameter("wsp", [NA, EMB], bf16, isOutput=False)
    bias_d = nc.declare_dram_parameter("bias", [128, 32], f32, isOutput=False)
    traj_d = nc.declare_dram_parameter("traj", [S, ROWS, 2], f32, isOutput=True)

    from contextlib import ExitStack

    with tile.TileContext(nc) as tc, ExitStack() as ctx:
        wp = ctx.enter_context(tc.tile_pool(name="wp", bufs=1))
        st = ctx.enter_context(tc.tile_pool(name="st", bufs=1))
        tmp = ctx.enter_context(tc.tile_pool(name="tmp", bufs=4))
        relp = ctx.enter_context(tc.tile_pool(name="relp", bufs=4))
        gp = ctx.enter_context(tc.tile_pool(name="gp", bufs=3, space="PSUM"))
        mp = ctx.enter_context(tc.tile_pool(name="mp", bufs=2, space="PSUM"))

        # --- resident weights/biases ---
        w0s = wp.tile([EMB, G3], bf16, tag="w0")
        nc.sync.dma_start(w0s[:], w0_d[:])
        w1s = []
        for k in range(KT):
            t = wp.tile([128, G3], bf16, tag=f"w1_{k}")
            nc.sync.dma_start(t[:], w1_d[k])
            w1s.append(t)
        wh2ps = []
        for k in range(KT):
            t = wp.tile([128, EMB], bf16, tag=f"wh2p_{k}")
            nc.sync.dma_start(t[:], wh2p_d[k])
            wh2ps.append(t)
        wm1s = wp.tile([EMB, MLP_H], bf16, tag="wm1")
        nc.sync.dma_start(wm1s[:], wm1_d[:])
        wm2s = wp.tile([MLP_H + 2, 2], bf16, tag="wm2")
        nc.sync.dma_start(wm2s[:], wm2_d[:])
        wsps = wp.tile([NA, EMB], bf16, tag="wsp")
        nc.sync.dma_start(wsps[:], wsp_d[:])
        bs_ = wp.tile([128, 32], f32, tag="bias")
        nc.sync.dma_start(bs_[:], bias_d[:])
        blks = st.tile([NA, ROWS], bf16, tag="blk")
        nc.sync.dma_start(blks[:], blk_d[:])

        # --- persistent state ---
        embT = st.tile([EMB, ROWS], bf16, tag="embT")
        h0T = [st.tile([128, ROWS], bf16, tag=f"h0T_{k}", name=f"h0T_{k}")
               for k in range(KT)]
        h1T = [st.tile([128, ROWS], bf16, tag=f"h1T_{k}", name=f"h1T_{k}")
               for k in range(KT)]
        tT = st.tile([MLP_H + 2, ROWS], bf16, tag="tT")
        nc.vector.memset(tT[MLP_H:MLP_H + 2, :], 1.0)  # ones rows for m2 bias

        # --- initial spatial embedding: embT = (blocks @ W_sp.T + b_sp).T ---
        for n in range(NCH):
            sl = slice(n * 512, (n + 1) * 512)
            ps0 = mp.tile([EMB, 512], f32, tag="mp", name=f"ps0_{n}")
            nc.tensor.matmul(ps0[:], wsps[:], blks[:, sl], start=True, stop=True)
            nc.vector.tensor_scalar_add(embT[:, sl], ps0[:], bs_[0:EMB, 24:25])

        def emit_tail(s):
            # --- t = relu(emb @ W_m1.T + b_m1); bias+relu fused on DVE ---
            for n in range(NCH):
                sl = slice(n * 512, (n + 1) * 512)
                ps_m = mp.tile([MLP_H, 512], f32, tag="mp", name=f"ps_m_{s}_{n}")
                nc.tensor.matmul(ps_m[:], wm1s[:], embT[:, sl],
                                 start=True, stop=True)
                nc.vector.tensor_scalar(tT[0:MLP_H, sl], ps_m[:],
                                        bs_[0:MLP_H, 26:27], 0.0,
                                        mybir.AluOpType.add, mybir.AluOpType.max)

            # --- rel = t @ W_m2.T + b_m2 (bias via ones rows), rows on psum.
            # All 8 row-tiles land in one psum tile, one copy, one DMA. ---
            m2big = gp.tile([128, 16], f32, tag="gps", name=f"m2big_{s}")
            for r in range(ROWS // 128):
                nc.tensor.matmul(m2big[:, r * 2:(r + 1) * 2],
                                 tT[:, r * 128:(r + 1) * 128], wm2s[:],
                                 start=True, stop=True)
            rstage = relp.tile([128, 16], f32, tag="rel")
            nc.vector.tensor_copy(rstage[:], m2big[:])
            out_ap = traj_d[s].rearrange("(r p) k -> p r k", p=128)
            nc.sync.dma_start(out_ap, rstage[:].rearrange("p (r k) -> p r k", k=2))

        for s in range(S):
            # --- 2-layer LSTM cell, zero state ---
            for layer in range(2):
                boff = 12 * layer
                hdst = h0T if layer == 0 else h1T

                def gate_mms(ps, gi, q):
                    # k outermost so one weight load covers both row chunks
                    col = (gi * 4 + q) * 128
                    if layer == 0:
                        for n in range(NCH):
                            sl = slice(n * 512, (n + 1) * 512)
                            nc.tensor.matmul(
                                ps[:, sl], w0s[:, col:col + 128], embT[:, sl],
                                start=True, stop=True)
                    else:
                        for k in range(KT):
                            for n in range(NCH):
                                sl = slice(n * 512, (n + 1) * 512)
                                nc.tensor.matmul(
                                    ps[:, sl], w1s[k][:, col:col + 128],
                                    h0T[k][:, sl],
                                    start=(k == 0), stop=(k == KT - 1))

                for q in range(4):
                    ps_i = gp.tile([128, ROWS], f32, tag="gps")
                    gate_mms(ps_i, 0, q)
                    ps_g = gp.tile([128, ROWS], f32, tag="gps")
                    gate_mms(ps_g, 1, q)
                    si = tmp.tile([128, ROWS], bf16, tag="si")
                    nc.scalar.activation(si[:], ps_i[:], AF.Sigmoid,
                                         bias=bs_[:, boff + q:boff + q + 1])
                    tg = tmp.tile([128, ROWS], bf16, tag="tg")
                    nc.scalar.activation(tg[:], ps_g[:], AF.Tanh,
                                         bias=bs_[:, boff + 4 + q:boff + 5 + q])
                    cc = tmp.tile([128, ROWS], bf16, tag="cc")
                    nc.vector.tensor_mul(cc[:], si[:], tg[:])
                    ps_o = gp.tile([128, ROWS], f32, tag="gps")
                    gate_mms(ps_o, 2, q)
                    so = tmp.tile([128, ROWS], bf16, tag="so")
                    nc.scalar.activation(so[:], ps_o[:], AF.Sigmoid,
                                         bias=bs_[:, boff + 8 + q:boff + 9 + q])
                    # h = tanh(c)*sig(o) via deg-5 odd poly on DVE:
                    # h = (A0 + A1 c^2 + A2 c^4) * c * sig(o)
                    c2 = tmp.tile([128, ROWS], bf16, tag="c2")
                    nc.vector.tensor_mul(c2[:], cc[:], cc[:])
                    pp = tmp.tile([128, ROWS], bf16, tag="pp")
                    nc.vector.tensor_scalar(pp[:], c2[:], TANH_A2, TANH_A1,
                                            mybir.AluOpType.mult,
                                            mybir.AluOpType.add)
                    vv = tmp.tile([128, ROWS], bf16, tag="vv")
                    nc.vector.tensor_mul(vv[:], pp[:], c2[:])
                    ww = tmp.tile([128, ROWS], bf16, tag="ww")
                    nc.vector.tensor_mul(ww[:], cc[:], so[:])
                    nc.vector.scalar_tensor_tensor(
                        hdst[q][:], vv[:], TANH_A0, ww[:],
                        mybir.AluOpType.add, mybir.AluOpType.mult)

            # --- emb = h1 @ W_h2p.T + b_h2p (carry); bias-add on DVE ---
            for n in range(NCH):
                sl = slice(n * 512, (n + 1) * 512)
                ps_e = mp.tile([EMB, 512], f32, tag="mp", name=f"ps_e_{s}_{n}")
                for k in range(KT):
                    nc.tensor.matmul(ps_e[:], wh2ps[k][:], h1T[k][:, sl],
                                     start=(k == 0), stop=(k == KT - 1))
                nc.vector.tensor_scalar_add(embT[:, sl], ps_e[:],
                                            bs_[0:EMB, 25:26])

            emit_tail(s)

    _split_excess_waits(nc)
    return nc


def _split_excess_waits(nc, max_waits=1):
    """walrus rejects instructions carrying more than one semaphore wait
    (seen on the Tile kernel-tail drain). Move excess waits onto preceding
    same-engine NOPs — engines execute in order, so semantics hold."""
    from concourse import mybir

    cnt = 0
    for f in nc.m.functions:
        for blk in f.blocks:
            il = list(blk.instructions)
            changed = False
            new = []
            for ins in il:
                si = getattr(ins, "sync_info", None)
                if si is not None and len(si.on_wait) > max_waits:
                    waits = list(si.on_wait)
                    while len(waits) > max_waits:
                        chunk, waits = waits[:max_waits], waits[max_waits:]
                        nop = mybir.InstNoOp(name=f"wsplit_{cnt}", ins=[], outs=[])
                        cnt += 1
                        nop.engine = ins.engine
                        nop.sync_info = mybir.SyncInfo(on_wait=chunk, on_update=[])
                        new.append(nop)
                    ins.sync_info = mybir.SyncInfo(
                        on_wait=waits, on_update=list(si.on_update))
                    changed = True
                new.append(ins)
            if changed:
                blk.instructions = new
    return cnt


def _install_ntff_hook():
    """The agent image lacks antenv.axon_hooks; synthesize it so
    run_bass_kernel_spmd(trace=True) can capture NTFF profiles."""
    import types

    if "antenv.axon_hooks" in sys.modules:
        return
    try:
        import trn_agent_boot.trn_boot as tb
        hook = tb._ntff_profile_via_ctypes("/opt/axon/libaxon_pjrt.so")
    except Exception:
        hook = None
    m = types.ModuleType("antenv.axon_hooks")
    m.get_axon_ntff_profile_hook = lambda: hook
    sys.modules["antenv.axon_hooks"] = m


def _prep_inputs(inputs):
    """Shard + lay out host-side: per-core input maps."""
    ind = np.asarray(inputs["input_data"], np.float32)
    n = ind.shape[0]
    assert n == B * NA, ind.shape
    idx = np.arange(B)
    blocks = ind.reshape(B, NA, B, NA)[idx, :, idx, :]  # [B, na, na]

    gate_cols = np.r_[0:H, 2 * H:4 * H]  # i, g, o rows of [4H, *] weights

    w0 = _bf(np.asarray(inputs["W_ih0"], np.float32).T[:, gate_cols])  # [64,1536]
    w1 = _bf(np.asarray(inputs["W_ih1"], np.float32).T[:, gate_cols]
              .reshape(KT, 128, G3))
    wh2p = _bf(np.asarray(inputs["W_h2p"], np.float32).T.reshape(KT, 128, EMB))
    wm1 = _bf(np.asarray(inputs["W_m1"], np.float32).T)  # [64,64]
    wsp = _bf(np.asarray(inputs["W_sp"], np.float32).T)  # [na,EMB]

    b_m2 = np.asarray(inputs["b_m2"], np.float32)
    bm2_hi = b_m2.astype(BF16).astype(np.float32)
    bm2_lo = b_m2 - bm2_hi
    wm2 = np.concatenate([
        np.asarray(inputs["W_m2"], np.float32).T,  # [64,2]
        bm2_hi[None, :], bm2_lo[None, :]], axis=0)
    wm2 = _bf(wm2)  # [66,2]

    bias = np.zeros((128, 32), np.float32)
    b0 = (np.asarray(inputs["b_ih0"], np.float32)
          + np.asarray(inputs["b_hh0"], np.float32))[gate_cols]
    b1 = (np.asarray(inputs["b_ih1"], np.float32)
          + np.asarray(inputs["b_hh1"], np.float32))[gate_cols]
    bias[:, 0:12] = b0.reshape(12, 128).T
    bias[:, 12:24] = b1.reshape(12, 128).T
    bias[0:EMB, 24] = np.asarray(inputs["b_sp"], np.float32)
    bias[0:EMB, 25] = np.asarray(inputs["b_h2p"], np.float32)
    bias[0:MLP_H, 26] = np.asarray(inputs["b_m1"], np.float32)

    shared = dict(w0=w0, w1=w1, wh2p=wh2p, wm1=wm1, wm2=wm2, wsp=wsp, bias=bias)
    in_maps = []
    for c in range(N_CORES):
        blk = blocks[c * BS_LOCAL:(c + 1) * BS_LOCAL]          # [16,na,na]
        blkT = _bf(blk.transpose(2, 0, 1).reshape(NA, ROWS))    # [na,1024]
        in_maps.append(dict(shared, blk=blkT))
    return in_maps


last_results = None


def kernel(input_data, num_agents, W_sp, b_sp, W_ih0, b_ih0, b_hh0,
           W_ih1, b_ih1, b_hh1, W_h2p, b_h2p, W_m1, b_m1, W_m2, b_m2):
    global last_results
    from concourse.bass_utils import run_bass_kernel_spmd

    inputs = dict(input_data=input_data, W_sp=W_sp, b_sp=b_sp,
                  W_ih0=W_ih0, b_ih0=b_ih0, b_hh0=b_hh0,
                  W_ih1=W_ih1, b_ih1=b_ih1, b_hh1=b_hh1,
                  W_h2p=W_h2p, b_h2p=b_h2p, W_m1=W_m1, b_m1=b_m1,
                  W_m2=W_m2, b_m2=b_m2)

    if "nc" not in _cache:
        _cache["nc"] = _build_program()
    nc = _cache["nc"]

    in_maps = _prep_inputs(inputs)
    trace = bool(int(os.environ.get("KERNEL_TRACE", "0")))
    if trace:
        _install_ntff_hook()
    res = run_bass_kernel_spmd(nc, in_maps, list(range(N_CORES)), trace=trace)
    last_results = res

    traj = np.empty((S, B * NA, 2), np.float32)
    for c in range(N_CORES):
        traj[:, c * ROWS:(c + 1) * ROWS, :] = res.results[c]["traj"]
    h_final = np.zeros((NL, NA, H), np.float32)
    return traj, h_final


# revision 25
# speedup vs baseline: 1.0726x; 1.0116x over previous
"""Trainium2 Bass kernel for nn_Decoder_13331578487330.

Model (reference semantics, all per batch-block):
  blocks[bs,na,na] -> x = blocks @ W_sp.T + b_sp        [bs,na,64]
  30x: 2-layer LSTM single step with zero initial state (so the f-gate
       contributes nothing: c = sig(i)*tanh(g), h = sig(o)*tanh(c)),
       emb = h @ W_h2p.T + b_h2p (the scan carry),
       rel = relu(emb @ W_m1.T + b_m1) @ W_m2.T + b_m2  -> traj[s]
  h_final is always zeros.

Sharding: data-parallel over the 128 diagonal blocks; 16 blocks (1024
rows) per NeuronCore, weights replicated. No cross-core communication.

On-chip layout: activations stored transposed ([channels, rows]) so
weights are the stationary matmul operand and per-channel biases land on
partitions (fused into ScalarE activation ops). Compute dtype bf16 with
fp32 PSUM accumulation (measured l2 rel err ~1e-3 vs fp32 reference).
"""

import os
import sys

import numpy as np

for _p in ("/opt/trn_rl_repo",):
    if _p not in sys.path and os.path.isdir(_p):
        sys.path.insert(0, _p)

import ml_dtypes

B, NA, S, H, EMB, MLP_H, NL = 128, 64, 30, 512, 64, 64, 2
N_CORES = 8
BS_LOCAL = B // N_CORES          # 16 diagonal blocks per core
ROWS = BS_LOCAL * NA             # 1024 rows per core
G3 = 3 * H                       # i,g,o gates only (f is dead)
QT = G3 // 128                   # 12 gate channel tiles of 128
KT = H // 128                    # 4 contraction tiles for H=512
NCH = ROWS // 512                # 2 row chunks of 512 (fp32 psum bank)

BF16 = ml_dtypes.bfloat16

# deg-5 odd minimax-ish fit of tanh on [-0.9, 0.9] (|c| <= ~0.54 in practice;
# max approx err 4.8e-4, below the bf16 noise floor)
TANH_A0, TANH_A1, TANH_A2 = 0.99861711, -0.31700099, 0.08317692

_cache = {}


def _bf(a):
    return np.ascontiguousarray(np.asarray(a, np.float32).astype(BF16))


def _build_program():
    import concourse.bass as bass
    import concourse.tile as tile
    from concourse import mybir

    f32 = mybir.dt.float32
    bf16 = mybir.dt.bfloat16
    AF = mybir.ActivationFunctionType

    nc = bass.Bass()

    blk_d = nc.declare_dram_parameter("blk", [NA, ROWS], bf16, isOutput=False)
    w0_d = nc.declare_dram_parameter("w0", [EMB, G3], bf16, isOutput=False)
    w1_d = nc.declare_dram_parameter("w1", [KT, 128, G3], bf16, isOutput=False)
    wh2p_d = nc.declare_dram_parameter("wh2p", [KT, 128, EMB], bf16, isOutput=False)
    wm1_d = nc.declare_dram_parameter("wm1", [EMB, MLP_H], bf16, isOutput=False)
    wm2_d = nc.declare_dram_parameter("wm2", [MLP_H + 2, 2], bf16, isOutput=False)
    wsp_d = nc.declare_dram_parameter("wsp", [NA, EMB], bf16, isOutput=False)
    bias_d = nc.declare_dram_parameter("bias", [128, 32], f32, isOutput=False)
    traj_d = nc.declare_dram_parameter("traj", [S, ROWS, 2], f32, isOutput=True)

    from contextlib import ExitStack

    with tile.TileContext(nc) as tc, ExitStack() as ctx:
        wp = ctx.enter_context(tc.tile_pool(name="wp", bufs=1))
        st = ctx.enter_context(tc.tile_pool(name="st", bufs=1))
        tmp = ctx.enter_context(tc.tile_pool(name="tmp", bufs=6))
        relp = ctx.enter_context(tc.tile_pool(name="relp", bufs=4))
        gp = ctx.enter_context(tc.tile_pool(name="gp", bufs=3, space="PSUM"))
        mp = ctx.enter_context(tc.tile_pool(name="mp", bufs=2, space="PSUM"))

        # --- resident weights/biases ---
        w0s = wp.tile([EMB, G3], bf16, tag="w0")
        nc.sync.dma_start(w0s[:], w0_d[:])
        w1s = []
        for k in range(KT):
            t = wp.tile([128, G3], bf16, tag=f"w1_{k}")
            nc.sync.dma_start(t[:], w1_d[k])
            w1s.append(t)
        wh2ps = []
        for k in range(KT):
            t = wp.tile([128, EMB], bf16, tag=f"wh2p_{k}")
            nc.sync.dma_start(t[:], wh2p_d[k])
            wh2ps.append(t)
        wm1s = wp.tile([EMB, MLP_H], bf16, tag="wm1")
        nc.sync.dma_start(wm1s[:], wm1_d[:])
        wm2s = wp.tile([MLP_H + 2, 2], bf16, tag="wm2")
        nc.sync.dma_start(wm2s[:], wm2_d[:])
        wsps = wp.tile([NA, EMB], bf16, tag="wsp")
        nc.sync.dma_start(wsps[:], wsp_d[:])
        bs_ = wp.tile([128, 32], f32, tag="bias")
        nc.sync.dma_start(bs_[:], bias_d[:])
        blks = st.tile([NA, ROWS], bf16, tag="blk")
        nc.sync.dma_start(blks[:], blk_d[:])

        # --- persistent state ---
        embT = st.tile([EMB, ROWS], bf16, tag="embT")
        h0T = [st.tile([128, ROWS], bf16, tag=f"h0T_{k}", name=f"h0T_{k}")
               for k in range(KT)]
        h1T = [st.tile([128, ROWS], bf16, tag=f"h1T_{k}", name=f"h1T_{k}")
               for k in range(KT)]
        tT = st.tile([MLP_H + 2, ROWS], bf16, tag="tT")
        nc.vector.memset(tT[MLP_H:MLP_H + 2, :], 1.0)  # ones rows for m2 bias

        # --- initial spatial embedding: embT = (blocks @ W_sp.T + b_sp).T ---
        for n in range(NCH):
            sl = slice(n * 512, (n + 1) * 512)
            ps0 = mp.tile([EMB, 512], f32, tag="mp", name=f"ps0_{n}")
            nc.tensor.matmul(ps0[:], wsps[:], blks[:, sl], start=True, stop=True)
            nc.vector.tensor_scalar_add(embT[:, sl], ps0[:], bs_[0:EMB, 24:25])

        def emit_tail(s):
            # --- t = relu(emb @ W_m1.T + b_m1); bias+relu fused on DVE ---
            for n in range(NCH):
                sl = slice(n * 512, (n + 1) * 512)
                ps_m = mp.tile([MLP_H, 512], f32, tag="mp", name=f"ps_m_{s}_{n}")
                nc.tensor.matmul(ps_m[:], wm1s[:], embT[:, sl],
                                 start=True, stop=True)
                nc.vector.tensor_scalar(tT[0:MLP_H, sl], ps_m[:],
                                        bs_[0:MLP_H, 26:27], 0.0,
                                        mybir.AluOpType.add, mybir.AluOpType.max)

            # --- rel = t @ W_m2.T + b_m2 (bias via ones rows), rows on psum.
            # All 8 row-tiles land in one psum tile, one copy, one DMA. ---
            m2big = mp.tile([128, 16], f32, tag="mp", name=f"m2big_{s}")
            for r in range(ROWS // 128):
                nc.tensor.matmul(m2big[:, r * 2:(r + 1) * 2],
                                 tT[:, r * 128:(r + 1) * 128], wm2s[:],
                                 start=True, stop=True)
            rstage = relp.tile([128, 16], f32, tag="rel")
            nc.vector.tensor_copy(rstage[:], m2big[:])
            out_ap = traj_d[s].rearrange("(r p) k -> p r k", p=128)
            nc.sync.dma_start(out_ap, rstage[:].rearrange("p (r k) -> p r k", k=2))

        for s in range(S):
            # --- 2-layer LSTM cell, zero state ---
            for layer in range(2):
                boff = 12 * layer
                hdst = h0T if layer == 0 else h1T

                def gate_mms(ps, gi, q):
                    # k outermost so one weight load covers both row chunks
                    col = (gi * 4 + q) * 128
                    if layer == 0:
                        for n in range(NCH):
                            sl = slice(n * 512, (n + 1) * 512)
                            nc.tensor.matmul(
                                ps[:, sl], w0s[:, col:col + 128], embT[:, sl],
                                start=True, stop=True)
                    else:
                        for k in range(KT):
                            for n in range(NCH):
                                sl = slice(n * 512, (n + 1) * 512)
                                nc.tensor.matmul(
                                    ps[:, sl], w1s[k][:, col:col + 128],
                                    h0T[k][:, sl],
                                    start=(k == 0), stop=(k == KT - 1))

                for q in range(4):
                    ps_i = gp.tile([128, ROWS], f32, tag="gps")
                    gate_mms(ps_i, 0, q)
                    ps_g = gp.tile([128, ROWS], f32, tag="gps")
                    gate_mms(ps_g, 1, q)
                    si = tmp.tile([128, ROWS], bf16, tag="si")
                    nc.scalar.activation(si[:], ps_i[:], AF.Sigmoid,
                                         bias=bs_[:, boff + q:boff + q + 1])
                    tg = tmp.tile([128, ROWS], bf16, tag="tg")
                    nc.scalar.activation(tg[:], ps_g[:], AF.Tanh,
                                         bias=bs_[:, boff + 4 + q:boff + 5 + q])
                    cc = tmp.tile([128, ROWS], bf16, tag="cc")
                    nc.vector.tensor_mul(cc[:], si[:], tg[:])
                    ps_o = gp.tile([128, ROWS], f32, tag="gps")
                    gate_mms(ps_o, 2, q)
                    so = tmp.tile([128, ROWS], bf16, tag="so")
                    nc.scalar.activation(so[:], ps_o[:], AF.Sigmoid,
                                         bias=bs_[:, boff + 8 + q:boff + 9 + q])
                    # h = tanh(c)*sig(o) via deg-5 odd poly on DVE:
                    # h = (A0 + A1 c^2 + A2 c^4) * c * sig(o)
                    c2 = tmp.tile([128, ROWS], bf16, tag="c2")
                    nc.vector.tensor_mul(c2[:], cc[:], cc[:])
                    pp = tmp.tile([128, ROWS], bf16, tag="pp")
                    nc.vector.tensor_scalar(pp[:], c2[:], TANH_A2, TANH_A1,
                                            mybir.AluOpType.mult,
                                            mybir.AluOpType.add)
                    vv = tmp.tile([128, ROWS], bf16, tag="vv")
                    nc.vector.tensor_mul(vv[:], pp[:], c2[:])
                    ww = tmp.tile([128, ROWS], bf16, tag="ww")
                    nc.vector.tensor_mul(ww[:], cc[:], so[:])
                    nc.vector.scalar_tensor_tensor(
                        hdst[q][:], vv[:], TANH_A0, ww[:],
                        mybir.AluOpType.add, mybir.AluOpType.mult)

            # --- emb = h1 @ W_h2p.T + b_h2p (carry); bias-add on DVE ---
            for n in range(NCH):
                sl = slice(n * 512, (n + 1) * 512)
                ps_e = mp.tile([EMB, 512], f32, tag="mp", name=f"ps_e_{s}_{n}")
                for k in range(KT):
                    nc.tensor.matmul(ps_e[:], wh2ps[k][:], h1T[k][:, sl],
                                     start=(k == 0), stop=(k == KT - 1))
                nc.vector.tensor_scalar_add(embT[:, sl], ps_e[:],
                                            bs_[0:EMB, 25:26])

            emit_tail(s)

    _split_excess_waits(nc)
    return nc


def _split_excess_waits(nc, max_waits=1):
    """walrus rejects instructions carrying more than one semaphore wait
    (seen on the Tile kernel-tail drain). Move excess waits onto preceding
    same-engine NOPs — engines execute in order, so semantics hold."""
    from concourse import mybir

    cnt = 0
    for f in nc.m.functions:
        for blk in f.blocks:
            il = list(blk.instructions)
            changed = False
            new = []
            for ins in il:
                si = getattr(ins, "sync_info", None)
                if si is not None and len(si.on_wait) > max_waits:
                    waits = list(si.on_wait)
                    while len(waits) > max_waits:
                        chunk, waits = waits[:max_waits], waits[max_waits:]
                        nop = mybir.InstNoOp(name=f"wsplit_{cnt}", ins=[], outs=[])
                        cnt += 1
                        nop.engine = ins.engine
                        nop.sync_info = mybir.SyncInfo(on_wait=chunk, on_update=[])
                        new.append(nop)
                    ins.sync_info = mybir.SyncInfo(
                        on_wait=waits, on_update=list(si.on_update))
                    changed = True
                new.append(ins)
            if changed:
                blk.instructions = new
    return cnt


def _install_ntff_hook():
    """The agent image lacks antenv.axon_hooks; synthesize it so
    run_bass_kernel_spmd(trace=True) can capture NTFF profiles."""
    import types

    if "antenv.axon_hooks" in sys.modules:
        return
    try:
        import trn_agent_boot.trn_boot as tb
        hook = tb._ntff_profile_via_ctypes("/opt/axon/libaxon_pjrt.so")
    except Exception:
        hook = None
    m = types.ModuleType("antenv.axon_hooks")
    m.get_axon_ntff_profile_hook = lambda: hook
    sys.modules["antenv.axon_hooks"] = m


def _prep_inputs(inputs):
    """Shard + lay out host-side: per-core input maps."""
    ind = np.asarray(inputs["input_data"], np.float32)
    n = ind.shape[0]
    assert n == B * NA, ind.shape
    idx = np.arange(B)
    blocks = ind.reshape(B, NA, B, NA)[idx, :, idx, :]  # [B, na, na]

    gate_cols = np.r_[0:H, 2 * H:4 * H]  # i, g, o rows of [4H, *] weights

    w0 = _bf(np.asarray(inputs["W_ih0"], np.float32).T[:, gate_cols])  # [64,1536]
    w1 = _bf(np.asarray(inputs["W_ih1"], np.float32).T[:, gate_cols]
              .reshape(KT, 128, G3))
    wh2p = _bf(np.asarray(inputs["W_h2p"], np.float32).T.reshape(KT, 128, EMB))
    wm1 = _bf(np.asarray(inputs["W_m1"], np.float32).T)  # [64,64]
    wsp = _bf(np.asarray(inputs["W_sp"], np.float32).T)  # [na,EMB]

    b_m2 = np.asarray(inputs["b_m2"], np.float32)
    bm2_hi = b_m2.astype(BF16).astype(np.float32)
    bm2_lo = b_m2 - bm2_hi
    wm2 = np.concatenate([
        np.asarray(inputs["W_m2"], np.float32).T,  # [64,2]
        bm2_hi[None, :], bm2_lo[None, :]], axis=0)
    wm2 = _bf(wm2)  # [66,2]

    bias = np.zeros((128, 32), np.float32)
    b0 = (np.asarray(inputs["b_ih0"], np.float32)
          + np.asarray(inputs["b_hh0"], np.float32))[gate_cols]
    b1 = (np.asarray(inputs["b_ih1"], np.float32)
          + np.asarray(inputs["b_hh1"], np.float32))[gate_cols]
    bias[:, 0:12] = b0.reshape(12, 128).T
    bias[:, 12:24] = b1.reshape(12, 128).T
    bias[0:EMB, 24] = np.asarray(inputs["b_sp"], np.float32)
    bias[0:EMB, 25] = np.asarray(inputs["b_h2p"], np.float32)
    bias[0:MLP_H, 26] = np.asarray(inputs["b_m1"], np.float32)

    shared = dict(w0=w0, w1=w1, wh2p=wh2p, wm1=wm1, wm2=wm2, wsp=wsp, bias=bias)
    in_maps = []
    for c in range(N_CORES):
        blk = blocks[c * BS_LOCAL:(c + 1) * BS_LOCAL]          # [16,na,na]
        blkT = _bf(blk.transpose(2, 0, 1).reshape(NA, ROWS))    # [na,1024]
        in_maps.append(dict(shared, blk=blkT))
    return in_maps


last_results = None


def kernel(input_data, num_agents, W_sp, b_sp, W_ih0, b_ih0, b_hh0,
           W_ih1, b_ih1, b_hh1, W_h2p, b_h2p, W_m1, b_m1, W_m2, b_m2):
    global last_results
    from concourse.bass_utils import run_bass_kernel_spmd

    inputs = dict(input_data=input_data, W_sp=W_sp, b_sp=b_sp,
                  W_ih0=W_ih0, b_ih0=b_ih0, b_hh0=b_hh0,
                  W_ih1=W_ih1, b_ih1=b_ih1, b_hh1=b_hh1,
                  W_h2p=W_h2p, b_h2p=b_h2p, W_m1=W_m1, b_m1=b_m1,
                  W_m2=W_m2, b_m2=b_m2)

    if "nc" not in _cache:
        _cache["nc"] = _build_program()
    nc = _cache["nc"]

    in_maps = _prep_inputs(inputs)
    trace = bool(int(os.environ.get("KERNEL_TRACE", "0")))
    if trace:
        _install_ntff_hook()
    res = run_bass_kernel_spmd(nc, in_maps, list(range(N_CORES)), trace=trace)
    last_results = res

    traj = np.empty((S, B * NA, 2), np.float32)
    for c in range(N_CORES):
        traj[:, c * ROWS:(c + 1) * ROWS, :] = res.results[c]["traj"]
    h_final = np.zeros((NL, NA, H), np.float32)
    return traj, h_final
